# revision 1
# baseline (speedup 1.0000x reference)
"""Trainium2 Bass kernel for nn_ContinuousEmbedding (histogram binning + distance-
weighted embedding mix).

Math: for each scalar x[b,f], the reference computes bucket index
idx = #{j in 1..63 : x > low[j]} and returns
    out[b,f,:] = sum_k weight[k,:] / (|idx-k|+1)  =  T[idx,:]
where T = S @ weight, S[i,k] = 1/(|i-k|+1) is a fixed 64x64 matrix.

T[idx] telescopes over compare signs s_j = sign(x - low[j]) (s_0 = +1 since
low[0] = -inf):
    T[idx] = sum_j s_j * V2[j],  V2[0] = (T[0]+T[63])/2, V2[j] = (T[j]-T[j-1])/2
i.e. out_row = V2^T s(x) -- a 64-deep contraction the TensorEngine runs with V2
as a permanently-resident stationary and the sign grid streaming as the moving
operand. The device output is the transposed [D, tokens] layout; the host
transposes once at unshard time.

Per 1024-token chunk (64 chunks per core, processed in chunk pairs):
  grid:   xb[64, 1024] = x broadcast to 64 partitions, via either
            gpsimd.partition_broadcast (SBUF)  -- element-rate-bound ~1.6us
          or PE rank-1 outer product ones[1,64]^T @ xrow (PSUM) -- ~0.43us
          (mix is tuned so GPSIMD and PE finish together)
  sign:   sg[64, 1024] fp16 = Sign(xb + (-low))   (ACT, per-partition bias)
  gather: ps[128, 512] psum, col-tiled: chunk A -> partitions 0:64 via
          tile_position (0,0), chunk B -> 64:128 via (0,64); both V2 copies
          stay resident in separate PE column groups (no LDWEIGHTS churn).
  copy:   DVE psum -> sbuf [128, 1024]
  out:    2 HWDGE DMAs -> outT[64, NTOK] rows (4KB contiguous runs)
V2/-low are precomputed on the host from weight/low in float64. Tokens whose x
exactly equals a bin edge (sign(0)=0) are patched exactly on the host.
"""

import os as _os
import sys

import numpy as np

for _p in ("/opt/trn_rl_repo",):
    if _p not in sys.path:
        sys.path.insert(0, _p)

import concourse.bass as bass  # noqa: E402,F401
import concourse.mybir as mybir  # noqa: E402
import concourse.tile as tile  # noqa: E402
from concourse import bacc  # noqa: E402
from concourse import bass_utils  # noqa: E402

B, F, K, D = 8192, 64, 64, 64
NCORES = 8
NTOK = (B // NCORES) * F          # 65536 tokens per core
CHUNK = 1024                      # tokens per chunk
NPAIR = NTOK // (2 * CHUNK)       # 32 chunk pairs
HALF = CHUNK // 2                 # tokens per matmul (N=512)

FP16 = mybir.dt.float16
F32 = mybir.dt.float32

CFG = {
    "pe_pairs_mod8": 5,   # of every 8 chunk pairs, this many use the PE grid path
}
for _kv in _os.environ.get("KCFG", "").split(","):
    if "=" in _kv:
        _k, _v = _kv.split("=", 1)
        CFG[_k.strip()] = int(_v) if _v.strip().lstrip("-").isdigit() else _v.strip()


def build_tile_kernel(nc, tc, x_d, low_d, v_d, out_d):
    x_ap = x_d.ap().rearrange("(c n) -> c n", c=NTOK // CHUNK)       # [64, 1024]
    out_ap = out_d.ap().rearrange("d (c n) -> c d n", c=NTOK // CHUNK)

    with tc.tile_pool(name="cpool", bufs=1) as cpool:
        neglow = cpool.tile([K, 1], F32)
        nc.sync.dma_start(out=neglow[:], in_=low_d.ap())
        vtab = cpool.tile([K, D], FP16)
        nc.sync.dma_start(out=vtab[:], in_=v_d.ap())
        ones = cpool.tile([1, K], F32)
        nc.vector.memset(ones[:], 1.0)

        with (
            tc.tile_pool(name="wpool", bufs=3) as wpool,
            tc.tile_pool(name="spool", bufs=4) as spool,
            tc.tile_pool(name="opool", bufs=3) as opool,
            tc.tile_pool(name="pxpool", bufs=2, space="PSUM") as pxpool,
            tc.tile_pool(name="popool", bufs=2, space="PSUM") as popool,
        ):
            for p in range(NPAIR):
                pe_path = (p % 8) < CFG["pe_pairs_mod8"]
                sgs = []
                for half in range(2):
                    c = 2 * p + half
                    xrow = wpool.tile([1, CHUNK], F32, tag="xrow", bufs=4)
                    nc.sync.dma_start(out=xrow[:], in_=x_ap[c])
                    sg = spool.tile([K, CHUNK], FP16, tag=f"sg{half}")
                    if pe_path:
                        xbp = pxpool.tile([K, CHUNK], F32, tag="xbp")
                        for h in range(2):
                            nc.tensor.matmul(
                                out=xbp[:, HALF * h : HALF * (h + 1)],
                                lhsT=ones[:],
                                rhs=xrow[:, HALF * h : HALF * (h + 1)],
                                start=True,
                                stop=True,
                            )
                        src = xbp
                    else:
                        xb = wpool.tile([K, CHUNK], F32, tag="xb")
                        nc.gpsimd.partition_broadcast(xb[:], xrow[:], channels=K)
                        src = xb
                    nc.scalar.activation(
                        out=sg[:],
                        in_=src[:],
                        func=mybir.ActivationFunctionType.Sign,
                        bias=neglow[:],
                        scale=1.0,
                    )
                    sgs.append(sg)

                ps = popool.tile([128, CHUNK], F32, tag="ps")
                for half in range(2):
                    for h in range(2):
                        nc.tensor.matmul(
                            out=ps[64 * half : 64 * (half + 1), HALF * h : HALF * (h + 1)],
                            lhsT=vtab[:],
                            rhs=sgs[half][:, HALF * h : HALF * (h + 1)],
                            start=True,
                            stop=True,
                            tile_position=(0, 64 * half),
                        )

                ob = opool.tile([128, CHUNK], F32, tag="ob")
                nc.vector.tensor_copy(out=ob[:], in_=ps[:])
                for half in range(2):
                    nc.sync.dma_start(
                        out=out_ap[2 * p + half],
                        in_=ob[64 * half : 64 * (half + 1), :],
                    )


_CACHED_NC = None


def _get_nc():
    global _CACHED_NC
    if _CACHED_NC is None:
        nc = bacc.Bacc("TRN2", target_bir_lowering=False, debug=False)
        x_d = nc.dram_tensor("x", [NTOK], F32, kind="ExternalInput")
        low_d = nc.dram_tensor("lowcol", [K, 1], F32, kind="ExternalInput")
        v_d = nc.dram_tensor("vtab", [K, D], FP16, kind="ExternalInput")
        out_d = nc.dram_tensor("out", [D, NTOK], F32, kind="ExternalOutput")
        with tile.TileContext(nc) as tc:
            build_tile_kernel(nc, tc, x_d, low_d, v_d, out_d)
        nc.compile()
        _CACHED_NC = nc
    return _CACHED_NC


def make_host_tables(low, weight):
    """V2 [K, D] fp16 (sign-telescoped table) and -low column [K, 1] f32,
    computed in float64."""
    ar = np.arange(K)
    S = 1.0 / (np.abs(ar[:, None] - ar[None, :]) + 1.0)              # [K, K] f64
    T = S @ np.asarray(weight, np.float64)                           # [K, D]
    V = np.empty_like(T)
    V[0] = (T[0] + T[-1]) / 2
    V[1:] = (T[1:] - T[:-1]) / 2
    vtab = V.astype(np.float16)
    lowcol = (-np.asarray(low, np.float64)).astype(np.float32).reshape(K, 1)
    return lowcol, vtab


def host_correct_ties(out2d, xflat, low, weight):
    """Exact fixup for tokens where x equals a bin edge: the device Sign gives
    sign(0)=0 there (averaging two table rows) while the reference uses strict
    x > low. Replace those few rows with the exact table row."""
    bins = np.asarray(low, np.float32)[1:]
    ties = np.isin(xflat, bins)
    if not ties.any():
        return out2d
    xt = xflat[ties]
    idx = (xt[:, None] > bins[None, :]).sum(-1)
    ar = np.arange(K)
    S = 1.0 / (np.abs(ar[:, None] - ar[None, :]) + 1.0)
    T = (S @ np.asarray(weight, np.float64)).astype(np.float32)
    out2d[ties] = T[idx]
    return out2d


def run_cores(x, low, weight, trace=False):
    """Shard, run on 8 cores, return ([NTOK*8, D] f32 output, BassKernelResults)."""
    lowcol, vtab = make_host_tables(low, weight)
    nc = _get_nc()
    shards = np.asarray(x, np.float32).reshape(NCORES, NTOK)
    in_maps = [
        {"x": np.ascontiguousarray(shards[i]), "lowcol": lowcol, "vtab": vtab}
        for i in range(NCORES)
    ]
    res = bass_utils.run_bass_kernel_spmd(
        nc, in_maps, core_ids=list(range(NCORES)), trace=trace
    )
    out = np.concatenate(
        [np.ascontiguousarray(res.results[i]["out"].T) for i in range(NCORES)], axis=0
    )
    return out, res


def kernel(x, low, high, weight):
    x = np.asarray(x, np.float32)
    out, _ = run_cores(x, low, weight)
    out = host_correct_ties(out, x.reshape(-1), low, weight)
    return out.reshape(B, F, D)



# revision 5
# speedup vs baseline: 1.7761x; 1.7761x over previous
"""Trainium2 Bass kernel for nn_ContinuousEmbedding (histogram binning + distance-
weighted embedding mix).

Math: for each scalar x[b,f], the reference computes bucket index
idx = #{j in 1..63 : x > low[j]} and returns
    out[b,f,:] = sum_k weight[k,:] / (|idx-k|+1)  =  T[idx,:]
where T = S @ weight, S[i,k] = 1/(|i-k|+1) is a fixed 64x64 matrix.

T[idx] telescopes over compare signs s_j = sign(x - low[j]) (s_0 = +1 since
low[0] = -inf):
    T[idx] = sum_j s_j * V2[j],  V2[0] = (T[0]+T[63])/2, V2[j] = (T[j]-T[j-1])/2
i.e. out_row = V2^T s(x) -- a 64-deep fp16 contraction on the TensorEngine.

Per 2048-token chunk pair (32 pairs per core), tiles are [128, 1024] with chunk
A on partitions 0:64 and chunk B on 64:128:
  grid:  xb[128, 1024] = x broadcast to 64 partitions per chunk, via one of
           - gpsimd.partition_broadcast
           - DMA with a stride-0 (replicated) DRAM read
           - PE outer product from an exact 3-way fp16 split of x shipped from
             the host (hx+mx+lx == x exactly in f32 accumulation), K=3 matmul
         (mix per 8 pairs is tuned so GPSIMD / DMA / PE finish together)
  sign:  sg[128, 1024] fp16 = Sign(xb + (-low))  (ACT, per-partition bias; for
         the PE path the bias-add reads the PSUM x directly)
  gather: 4 matmuls vtab^T sg -> psum [128, 1024], diagonal PE quadrants
          (0,0)/(64,64) so both chunks' grids gather without moving data
  out:   DVE psum -> sbuf fp16 [128, 1024]; 2 HWDGE DMAs -> outT[64, NTOK]
The device output is transposed [D, tokens] fp16; the host transposes + casts
to f32 once at unshard time. Ties (x exactly equal to a bin edge) give
sign(0)=0 and are patched exactly on the host, as before.
"""

import os as _os
import sys

import numpy as np

for _p in ("/opt/trn_rl_repo",):
    if _p not in sys.path:
        sys.path.insert(0, _p)

import concourse.bass as bass  # noqa: E402,F401
import concourse.mybir as mybir  # noqa: E402
import concourse.tile as tile  # noqa: E402
from concourse import bacc  # noqa: E402
from concourse import bass_utils  # noqa: E402

B, F, K, D = 8192, 64, 64, 64
NCORES = 8
NTOK = (B // NCORES) * F          # 65536 tokens per core
CHUNK = 1024                      # tokens per chunk
NPAIR = NTOK // (2 * CHUNK)       # 32 chunk pairs
HALF = CHUNK // 2                 # tokens per matmul (N=512)

FP16 = mybir.dt.float16
F32 = mybir.dt.float32

CFG = {
    # of every 8 chunk pairs: first ngp use GPSIMD broadcast, next ndma use
    # DMA stride-0 broadcast, rest use the PE fp16-split outer product.
    "ngp": 4,
    "ndma": 3,
}
for _kv in _os.environ.get("KCFG", "").split(","):
    if "=" in _kv:
        _k, _v = _kv.split("=", 1)
        CFG[_k.strip()] = int(_v) if _v.strip().lstrip("-").isdigit() else _v.strip()

SIGN = mybir.ActivationFunctionType.Sign


def build_tile_kernel(nc, tc, x_d, xs_d, low_d, v_d, out_d):
    x_ap = x_d.ap().rearrange("(c n) -> c n", c=NTOK // CHUNK)       # [64, 1024]
    xs_ap = xs_d.ap().rearrange("k (p n) -> p k n", p=NPAIR)         # [32, 3, 2048]
    out_ap = out_d.ap().rearrange("d (c n) -> c d n", c=NTOK // CHUNK)

    ngp, ndma = CFG["ngp"], CFG["ndma"]

    with tc.tile_pool(name="cpool", bufs=1) as cpool:
        neglow = cpool.tile([K, 1], F32)
        nc.sync.dma_start(out=neglow[:], in_=low_d.ap())
        vtab = cpool.tile([K, D], FP16)
        nc.sync.dma_start(out=vtab[:], in_=v_d.ap())
        ones3 = cpool.tile([3, K], FP16)
        nc.vector.memset(ones3[:], 1.0)

        with (
            tc.tile_pool(name="wpool", bufs=3) as wpool,
            tc.tile_pool(name="spool", bufs=3) as spool,
            tc.tile_pool(name="opool", bufs=3) as opool,
            tc.tile_pool(name="pxpool", bufs=1, space="PSUM") as pxpool,
            tc.tile_pool(name="popool", bufs=2, space="PSUM") as popool,
        ):
            for p in range(NPAIR):
                r = p % 8
                mech = "gp" if r < ngp else ("dma" if r < ngp + ndma else "pe")
                c0, c1 = 2 * p, 2 * p + 1
                sg = spool.tile([K, 2 * CHUNK], FP16, tag="sg")

                if mech == "pe":
                    xsp = wpool.tile([3, 2 * CHUNK], FP16, tag="xsp")
                    nc.sync.dma_start(out=xsp[:], in_=xs_ap[p])
                    px = pxpool.tile([K, 2 * CHUNK], F32, tag="px")
                    for h in range(4):
                        sl = slice(HALF * h, HALF * (h + 1))
                        nc.tensor.matmul(
                            out=px[:, sl], lhsT=ones3[:], rhs=xsp[:, sl],
                            start=True, stop=True, tile_position=(0, 0),
                        )
                    src = px
                else:
                    xb = wpool.tile([K, 2 * CHUNK], F32, tag="xb")
                    if mech == "gp":
                        xr0 = wpool.tile([1, CHUNK], F32, tag="xr0")
                        xr1 = wpool.tile([1, CHUNK], F32, tag="xr1")
                        nc.sync.dma_start(out=xr0[:], in_=x_ap[c0])
                        nc.sync.dma_start(out=xr1[:], in_=x_ap[c1])
                        nc.gpsimd.partition_broadcast(xb[:, 0:CHUNK], xr0[:], channels=K)
                        nc.gpsimd.partition_broadcast(
                            xb[:, CHUNK : 2 * CHUNK], xr1[:], channels=K
                        )
                    else:
                        nc.sync.dma_start(
                            out=xb[:, 0:CHUNK],
                            in_=x_ap[c0].unsqueeze(0).broadcast_to([K, CHUNK]),
                        )
                        nc.sync.dma_start(
                            out=xb[:, CHUNK : 2 * CHUNK],
                            in_=x_ap[c1].unsqueeze(0).broadcast_to([K, CHUNK]),
                        )
                    src = xb
                nc.scalar.activation(
                    out=sg[:], in_=src[:], func=SIGN, bias=neglow[:], scale=1.0
                )

                po = popool.tile([128, CHUNK], F32, tag="po")
                for h in range(2):
                    sl = slice(HALF * h, HALF * (h + 1))
                    nc.tensor.matmul(
                        out=po[0:64, sl], lhsT=vtab[:], rhs=sg[:, sl],
                        start=True, stop=True, tile_position=(0, 0),
                    )
                    nc.tensor.matmul(
                        out=po[64:128, sl], lhsT=vtab[:],
                        rhs=sg[:, CHUNK + HALF * h : CHUNK + HALF * (h + 1)],
                        start=True, stop=True, tile_position=(0, 64),
                    )

                ob = opool.tile([128, CHUNK], FP16, tag="ob")
                nc.vector.tensor_copy(out=ob[:], in_=po[:])
                nc.sync.dma_start(out=out_ap[c0], in_=ob[0:64, :])
                nc.sync.dma_start(out=out_ap[c1], in_=ob[64:128, :])


_CACHED_NC = None


def _get_nc():
    global _CACHED_NC
    if _CACHED_NC is None:
        nc = bacc.Bacc("TRN2", target_bir_lowering=False, debug=False)
        x_d = nc.dram_tensor("x", [NTOK], F32, kind="ExternalInput")
        xs_d = nc.dram_tensor("xsplit", [3, NTOK], FP16, kind="ExternalInput")
        low_d = nc.dram_tensor("lowcol", [K, 1], F32, kind="ExternalInput")
        v_d = nc.dram_tensor("vtab", [K, D], FP16, kind="ExternalInput")
        out_d = nc.dram_tensor("out", [D, NTOK], FP16, kind="ExternalOutput")
        with tile.TileContext(nc) as tc:
            build_tile_kernel(nc, tc, x_d, xs_d, low_d, v_d, out_d)
        nc.compile()
        _CACHED_NC = nc
    return _CACHED_NC


def make_host_tables(low, weight):
    """V2 duplicated to [128, D] fp16 (sign-telescoped table) and -low column
    duplicated to [128, 1] f32, computed in float64."""
    ar = np.arange(K)
    S = 1.0 / (np.abs(ar[:, None] - ar[None, :]) + 1.0)              # [K, K] f64
    T = S @ np.asarray(weight, np.float64)                           # [K, D]
    V = np.empty_like(T)
    V[0] = (T[0] + T[-1]) / 2
    V[1:] = (T[1:] - T[:-1]) / 2
    vtab = V.astype(np.float16)
    lowcol = (-np.asarray(low, np.float64)).astype(np.float32).reshape(K, 1)
    return lowcol, vtab


def split_fp16_3(xflat):
    """Exact 3-way fp16 split: hx + mx + lx == x in f32 (24 <= 3*11 mantissa
    bits; each partial sum is exactly representable)."""
    x = np.asarray(xflat, np.float32)
    hx = x.astype(np.float16)
    r = x - hx.astype(np.float32)
    mx = r.astype(np.float16)
    r2 = r - mx.astype(np.float32)
    lx = r2.astype(np.float16)
    return np.stack([hx, mx, lx], axis=0)                            # [3, NTOK]


def host_correct_ties(out2d, xflat, low, weight):
    """Exact fixup for tokens where x equals a bin edge: the device Sign gives
    sign(0)=0 there (averaging two table rows) while the reference uses strict
    x > low. Replace those few rows with the exact table row."""
    bins = np.asarray(low, np.float32)[1:]
    ties = np.isin(xflat, bins)
    if not ties.any():
        return out2d
    xt = xflat[ties]
    idx = (xt[:, None] > bins[None, :]).sum(-1)
    ar = np.arange(K)
    S = 1.0 / (np.abs(ar[:, None] - ar[None, :]) + 1.0)
    T = (S @ np.asarray(weight, np.float64)).astype(np.float32)
    out2d[ties] = T[idx]
    return out2d


def make_in_maps(x, low, weight):
    lowcol2, vtab2 = make_host_tables(low, weight)
    shards = np.asarray(x, np.float32).reshape(NCORES, NTOK)
    in_maps = []
    for i in range(NCORES):
        xi = np.ascontiguousarray(shards[i])
        in_maps.append(
            {"x": xi, "xsplit": split_fp16_3(xi), "lowcol": lowcol2, "vtab": vtab2}
        )
    return in_maps


def run_cores(x, low, weight, trace=False):
    """Shard, run on 8 cores, return ([NTOK*8, D] f32 output, BassKernelResults)."""
    nc = _get_nc()
    in_maps = make_in_maps(x, low, weight)
    res = bass_utils.run_bass_kernel_spmd(
        nc, in_maps, core_ids=list(range(NCORES)), trace=trace
    )
    out = np.concatenate(
        [
            np.ascontiguousarray(res.results[i]["out"].T.astype(np.float32))
            for i in range(NCORES)
        ],
        axis=0,
    )
    return out, res


def kernel(x, low, high, weight):
    x = np.asarray(x, np.float32)
    out, _ = run_cores(x, low, weight)
    out = host_correct_ties(out, x.reshape(-1), low, weight)
    return out.reshape(B, F, D)
